# revision 1
# baseline (speedup 1.0000x reference)
"""PointPillarScatter Trainium2 kernel.

Strategy: shard by (batch, y-half) -> 8 cores, each producing a
[64, 107136] channel-major slab of the BEV grid.

The scatter+transpose+zero-fill is fused into per-tile PE matmuls:
for each 512-cell tile, out[64, 512] = feat_tile[K, 64]^T @ onehot[K, 512]
where onehot[k, j] = (cell_offset_k == j) is built on DVE via is_equal
against an iota row. PSUM start=True writes zeros for empty cells, so no
separate zero-fill pass is needed, and the output is written exactly once,
densely.

Host prep: last-write-wins dedup of duplicate cells (matches the
reference's scatter semantics), bucketing pillars by (core, tile), and
padding each tile's pillar list to a fixed K_pad.
"""

import numpy as np

B, C, NY, NX = 4, 64, 496, 432
CELLS_B = NY * NX          # 214272 cells per batch
HALF = CELLS_B // 2        # 107136 cells per core slab
N_CORES = 8
TILE_N = 512               # cells per matmul tile (one PSUM bank of f32)
N_TILES = (HALF + TILE_N - 1) // TILE_N   # 210 (tile 209 has only 128 cells)
TAIL_N = HALF - (N_TILES - 1) * TILE_N    # 128
GROUP_CELLS = 2048         # cells per PSUM group
SG_CELLS = 4096            # cells per onehot/stage/output-store super-group
N_GROUPS = (HALF + GROUP_CELLS - 1) // GROUP_CELLS  # 53 (last group 640 cells)
CHUNK_T = 32               # feature tiles per DMA chunk (group-aligned)


def make_iota():
    """[128, SG_CELLS] f32, 0..TILE_N-1 repeated per tile span."""
    row = np.tile(np.arange(TILE_N, dtype=np.float32), SG_CELLS // TILE_N)
    return np.broadcast_to(row[None, :], (128, SG_CELLS)).copy()


def _host_prep(pf, vc):
    """Dedup (last-wins), shard, bucket and pad pillars.

    Returns featT [N_CORES, K_pad, N_TILES*64] f32,
            offs  [N_CORES, K_pad, N_TILES] f32 (pad = -1),
            K_pad.
    """
    pf = np.ascontiguousarray(np.asarray(pf, dtype=np.float32))
    vc = np.asarray(vc)
    b = vc[:, 0].astype(np.int64)
    y = vc[:, 2].astype(np.int64)
    x = vc[:, 3].astype(np.int64)
    cell = y * NX + x
    key = b * CELLS_B + cell

    # last occurrence of each key wins (matches reference scatter)
    u, idx_rev = np.unique(key[::-1], return_index=True)
    winners = (len(key) - 1) - idx_rev

    wb = u // CELLS_B
    wc = u % CELLS_B
    h = (wc >= HALF).astype(np.int64)
    core = wb * 2 + h
    cl = wc - h * HALF
    tile = cl // TILE_N
    off = cl % TILE_N

    gkey = core * N_TILES + tile
    order = np.argsort(gkey, kind="stable")
    gk_s = gkey[order]
    starts = np.r_[0, np.flatnonzero(np.diff(gk_s)) + 1]
    counts = np.diff(np.r_[starts, len(gk_s)])
    K_pad = max(16, int(np.ceil(counts.max() / 16) * 16))

    rank = np.arange(len(gk_s)) - np.repeat(starts, counts)
    w_s = winners[order]
    core_s = core[order]
    tile_s = tile[order]
    off_s = off[order]

    featT = np.zeros((N_CORES, K_pad, N_TILES, 64), np.float32)
    offs = np.full((N_CORES, K_pad, N_TILES), -1.0, np.float32)
    featT[core_s, rank, tile_s, :] = pf[w_s]
    offs[core_s, rank, tile_s] = off_s
    return featT.reshape(N_CORES, K_pad, N_TILES * 64), offs, K_pad


def _sim_core(featT_c, offs_c, K_pad):
    """Numpy simulation of one core's device program (for validation)."""
    out = np.zeros((64, HALF), np.float32)
    fv = featT_c.reshape(K_pad, N_TILES, 64)
    for t in range(N_TILES):
        n = TILE_N if t < N_TILES - 1 else TAIL_N
        oh = (offs_c[:, t : t + 1] == np.arange(n)[None, :]).astype(np.float32)
        out[:, t * TILE_N : t * TILE_N + n] = fv[:, t, :].T @ oh
    return out


def _build_bass(K_pad, repeat=1):
    import concourse.bacc as bacc
    import concourse.bass as bass
    import concourse.tile as tile
    from concourse import mybir
    from contextlib import ExitStack

    f32 = mybir.dt.float32
    nc = bacc.Bacc("TRN2", target_bir_lowering=False, debug=False)

    featT = nc.dram_tensor("featT", [K_pad, N_TILES * 64], f32, kind="ExternalInput")
    offs = nc.dram_tensor("offs", [K_pad, N_TILES], f32, kind="ExternalInput")
    iota = nc.dram_tensor("iota", [128, SG_CELLS], f32, kind="ExternalInput")
    out = nc.dram_tensor("out", [64, HALF], f32, kind="ExternalOutput")

    with tile.TileContext(nc) as tc, ExitStack() as ctx:
        const_p = ctx.enter_context(tc.tile_pool(name="const", bufs=1))
        feat_p = ctx.enter_context(tc.tile_pool(name="feat", bufs=3))
        oh_p = ctx.enter_context(tc.tile_pool(name="oh", bufs=8))
        ps_p = ctx.enter_context(tc.tile_pool(name="ps", bufs=2, space="PSUM"))
        st_p = ctx.enter_context(tc.tile_pool(name="st", bufs=6))

        iota_t = const_p.tile([K_pad, GROUP_CELLS], f32)
        nc.sync.dma_start(out=iota_t[:], in_=iota[:K_pad, :GROUP_CELLS])
        off_t = const_p.tile([K_pad, N_TILES], f32)
        nc.gpsimd.dma_start(out=off_t[:], in_=offs[:, :])

        def body():
            feat_chunk = None
            for g in range(N_GROUPS):
                g_lo = g * GROUP_CELLS
                g_hi = min(g_lo + GROUP_CELLS, HALF)
                g_n = g_hi - g_lo
                psum = ps_p.tile([64, g_n], f32, tag="ps")
                t0 = g_lo // TILE_N
                n_sub = (g_n + TILE_N - 1) // TILE_N

                if t0 % CHUNK_T == 0:
                    w = min(CHUNK_T, N_TILES - t0)
                    feat_chunk = feat_p.tile([K_pad, w * 64], f32, tag="feat")
                    # separate HWDGE ring (ACT) so chunk prefetches don't
                    # queue behind the output stores on the SP ring
                    nc.scalar.dma_start(
                        out=feat_chunk[:],
                        in_=featT[:, t0 * 64 : (t0 + w) * 64],
                    )

                oh = oh_p.tile([K_pad, g_n], f32, tag="oh")
                n_full = g_n // TILE_N
                if n_full:
                    nc.vector.tensor_tensor(
                        out=oh[:, : n_full * TILE_N],
                        in0=off_t[:, t0 : t0 + n_full].to_broadcast(
                            [K_pad, n_full, TILE_N]
                        ),
                        in1=iota_t[:K_pad, : n_full * TILE_N],
                        op=mybir.AluOpType.is_equal,
                    )
                if g_n > n_full * TILE_N:  # ragged tail tile
                    n = g_n - n_full * TILE_N
                    nc.vector.tensor_tensor(
                        out=oh[:, n_full * TILE_N :],
                        in0=off_t[:, t0 + n_full : t0 + n_full + 1].to_broadcast(
                            [K_pad, n]
                        ),
                        in1=iota_t[:K_pad, :n],
                        op=mybir.AluOpType.is_equal,
                    )

                for s in range(n_sub):
                    t = t0 + s
                    n = min(TILE_N, g_n - s * TILE_N)
                    j = t % CHUNK_T
                    nc.tensor.matmul(
                        out=psum[:, s * TILE_N : s * TILE_N + n],
                        lhsT=feat_chunk[:, j * 64 : (j + 1) * 64],
                        rhs=oh[:, s * TILE_N : s * TILE_N + n],
                        is_transpose=True,
                        start=True,
                        stop=True,
                    )
                stage = st_p.tile([64, g_n], f32, tag="st")
                nc.scalar.copy(out=stage[:], in_=psum[:])
                nc.sync.dma_start(out=out[:, g_lo:g_hi], in_=stage[:])

        if repeat == 1:
            body()
        else:
            with tc.For_i(0, repeat, 1):
                body()

    nc.compile()
    return nc


def _run(pillar_features, voxel_coords, trace=False, prep=None):
    featT, offs, K_pad = (
        prep if prep is not None else _host_prep(pillar_features, voxel_coords)
    )
    iota = make_iota()

    from concourse.bass_utils import run_bass_kernel_spmd

    nc = _build_bass(K_pad)
    in_maps = [
        {"featT": featT[c], "offs": offs[c], "iota": iota} for c in range(N_CORES)
    ]
    res = run_bass_kernel_spmd(
        nc, in_maps, core_ids=list(range(N_CORES)), trace=trace
    )

    out_full = np.empty((B, C, CELLS_B), np.float32)
    for core in range(N_CORES):
        bb, h = core // 2, core % 2
        out_full[bb, :, h * HALF : (h + 1) * HALF] = res.results[core]["out"]
    return out_full.reshape(B, C, NY, NX), res


def kernel(pillar_features, voxel_coords):
    featT, offs, K_pad = _host_prep(pillar_features, voxel_coords)
    if K_pad > 128:
        # PE matmul K is capped at 128 partitions; with the given input
        # distribution K_pad is ~80, so this path is never taken. Kept as
        # a correctness safety net.
        out_full = np.empty((B, C, CELLS_B), np.float32)
        for core in range(N_CORES):
            bb, h = core // 2, core % 2
            out_full[bb, :, h * HALF : (h + 1) * HALF] = _sim_core(
                featT[core], offs[core], K_pad
            )
        return out_full.reshape(B, C, NY, NX)
    return _run(
        pillar_features, voxel_coords, trace=False, prep=(featT, offs, K_pad)
    )[0]


def profile_hw(pillar_features, voxel_coords):
    _, res = _run(pillar_features, voxel_coords, trace=True)
    return res.exec_time_ns


if __name__ == "__main__":
    # quick numpy-sim self check against last-wins reference
    rng = np.random.default_rng(0)
    n = 20000
    pf = rng.standard_normal((n, 64)).astype(np.float32)
    vc = np.stack(
        [
            rng.integers(0, B, n),
            np.zeros(n, np.int64),
            rng.integers(0, NY, n),
            rng.integers(0, NX, n),
        ],
        axis=1,
    ).astype(np.int64)
    featT, offs, K_pad = _host_prep(pf, vc)
    print("K_pad =", K_pad)
    # last-wins reference
    grid = np.zeros((B * CELLS_B, 64), np.float32)
    flat = vc[:, 0] * CELLS_B + vc[:, 2] * NX + vc[:, 3]
    grid[flat] = pf
    ref = grid.reshape(B, CELLS_B, 64).transpose(0, 2, 1)
    for core in range(N_CORES):
        bb, h = core // 2, core % 2
        slab = _sim_core(featT[core], offs[core], K_pad)
        exp = ref[bb, :, h * HALF : (h + 1) * HALF]
        assert np.array_equal(slab, exp), f"core {core} mismatch"
    print("numpy sim matches last-wins reference")

